# revision 2
# baseline (speedup 1.0000x reference)
"""GQA attention block on 8 NeuronCores.

Sharding: tensor-parallel over head groups (4 ways: 8 q heads / 2 kv heads
per core) x data-parallel over batch (2 ways).  Each core computes a partial
y = attn_out_slice @ Wo_slice for its (batch, head-group); the host sums the
4 TP partials per batch element.

Per-core device program (all fp32):
  A) x^T via PE transposes; q^T/k^T/v^T projections (q scaled by 1/sqrt(dh)).
  B) per head: S^T tiles = k^T.T @ q^T, exp on ACT (no max subtraction --
     inputs are scaled gaussians, |S|<~6 so exp is safe in fp32), then
     PV via lhsT=[v|ones]: rows 0..63 accumulate unnormalized out^T, row 64
     accumulates the softmax denominator.  Normalize with a reciprocal +
     partition-broadcast + multiply.
  C) y = out^T.T @ Wo.
"""

import os
import sys

import numpy as np

for _p in ("/opt/trn_rl_repo",):
    if os.path.isdir(_p) and _p not in sys.path:
        sys.path.insert(0, _p)

from contextlib import ExitStack

import concourse.bass as bass  # noqa: F401  (AP types pulled in transitively)
import concourse.mybir as mybir
import concourse.tile as tile
from concourse import bacc
from concourse.bass_utils import run_bass_kernel_spmd
from concourse.masks import make_identity

P = 128
B, T, D = 2, 2048, 2048
HQ, HKV, DH = 32, 8, 64
GROUP = HQ // HKV            # 4
TP = 4                       # tensor-parallel ways
DP = 2                       # data-parallel ways
NCORES = TP * DP
DQ = D // TP                 # 512 q dims per core (8 heads)
DKV = HKV * DH // TP         # 128 kv dims per core (2 kv heads)
NHQ = HQ // TP               # 8 q heads per core
NKV = HKV // TP              # 2 kv heads per core
NKS = D // P                 # 16 contraction subtiles over D
NT = T // P                  # 16 row tiles over T
CH = 512                     # T chunk width in projection phase
NCH = T // CH                # 4
TQB = 512                    # T_q block width in attention / psum bank
NTQB = T // TQB              # 4
NKI = T // P                 # 16 key tiles
NB = D // 512                # 4 output column banks
SCALE = 1.0 / 8.0            # 1/sqrt(DH)
F32 = mybir.dt.float32
F32R = mybir.dt.float32r
AF = mybir.ActivationFunctionType


def _build():
    nc = bacc.Bacc(None, target_bir_lowering=False, debug=False)

    x_ext = nc.dram_tensor("x", [T, D], F32, kind="ExternalInput")
    wq_ext = nc.dram_tensor("wq", [D, DQ], F32, kind="ExternalInput")
    wk_ext = nc.dram_tensor("wk", [D, DKV], F32, kind="ExternalInput")
    wv_ext = nc.dram_tensor("wv", [D, DKV], F32, kind="ExternalInput")
    wo_ext = nc.dram_tensor("wo", [DQ, D], F32, kind="ExternalInput")
    y_ext = nc.dram_tensor("y", [T, D], F32, kind="ExternalOutput")

    x_v = x_ext[:].rearrange("(to p) d -> p to d", p=P)      # [128,16,2048]
    wq_v = wq_ext[:].rearrange("(ko p) m -> p ko m", p=P)    # [128,16,512]
    wk_v = wk_ext[:].rearrange("(ko p) m -> p ko m", p=P)    # [128,16,128]
    wv_v = wv_ext[:].rearrange("(ko p) m -> p ko m", p=P)
    wo_v = wo_ext[:].rearrange("(ko p) n -> p ko n", p=P)    # [128,4,2048]
    y_v = y_ext[:].rearrange("(to p) n -> p to n", p=P)      # [128,16,2048]

    with tile.TileContext(nc) as tc, ExitStack() as ctx:
        const = ctx.enter_context(tc.tile_pool(name="const", bufs=1))
        big = ctx.enter_context(tc.tile_pool(name="big", bufs=3))
        wkv_p = ctx.enter_context(tc.tile_pool(name="wkv", bufs=1))
        row_p = ctx.enter_context(tc.tile_pool(name="rows", bufs=2))
        qt_p = ctx.enter_context(tc.tile_pool(name="qt", bufs=1))
        kt_p = ctx.enter_context(tc.tile_pool(name="kt", bufs=1))
        vo_p = ctx.enter_context(tc.tile_pool(name="vo", bufs=1))
        exp_p = ctx.enter_context(tc.tile_pool(name="expp", bufs=2))
        bc_p = ctx.enter_context(tc.tile_pool(name="bcp", bufs=2))
        rc_p = ctx.enter_context(tc.tile_pool(name="rcp", bufs=2))
        ot_p = ctx.enter_context(tc.tile_pool(name="otp", bufs=1))

        aux_ps = ctx.enter_context(tc.tile_pool(name="aux_ps", bufs=2, space="PSUM"))
        proj_ps = ctx.enter_context(tc.tile_pool(name="proj_ps", bufs=2, space="PSUM"))
        s_ps = ctx.enter_context(tc.tile_pool(name="s_ps", bufs=2, space="PSUM"))
        pv_ps = ctx.enter_context(tc.tile_pool(name="pv_ps", bufs=2, space="PSUM"))

        identity = const.tile([P, P], F32)
        make_identity(nc, identity)
        ones1 = const.tile([1, DH], F32)
        nc.gpsimd.memset(ones1[:], 1.0)

        wq_sb = big.tile([P, NKS, DQ], F32R, tag="big")
        wk_sb = wkv_p.tile([P, NKS, DKV], F32R, tag="wk")
        wv_sb = wkv_p.tile([P, NKS, DKV], F32R, tag="wv")

        qt_sb = qt_p.tile([P, DQ // P, T], F32R)        # q^T * SCALE, [dim, t]
        kt_sb = kt_p.tile([P, T], F32R)                 # k^T, [dim(2 kv heads), t]
        vones = vo_p.tile([P, NKV, NKI, DH + 1], F32R)  # [t%128, kv, t//128, dh|1]
        ones_col = const.tile([P, NKV, NKI], F32)
        nc.gpsimd.memset(ones_col[:], 1.0)
        nc.vector.tensor_copy(vones[:, :, :, DH], ones_col[:])

        # ---- Phase A: x^T chunks + projections ----
        for c in range(NCH):
            xt_ch = big.tile([P, NKS, CH], F32R, tag="big")  # x^T[:, c*CH:+CH]
            for r in range(CH // P):
                xrow = row_p.tile([P, D], F32, tag="rows")
                nc.sync.dma_start(xrow[:], x_v[:, c * (CH // P) + r, :])
                for dsb in range(NKS):
                    tp = aux_ps.tile([P, P], F32, tag="aux")
                    nc.tensor.transpose(tp[:], xrow[:, dsb * P:(dsb + 1) * P], identity)
                    nc.vector.tensor_copy(xt_ch[:, dsb, r * P:(r + 1) * P], tp[:])
            if c == 0:
                # weights go out after the first x rows so PE transposes
                # start as early as possible
                nc.sync.dma_start(wq_sb[:], wq_v.bitcast(F32R))
                nc.sync.dma_start(wk_sb[:], wk_v.bitcast(F32R))
                nc.sync.dma_start(wv_sb[:], wv_v.bitcast(F32R))
            # q^T chunk, scaled by 1/sqrt(dh) on eviction
            for mb in range(DQ // P):
                qp = proj_ps.tile([P, CH], F32, tag="proj")
                for ks in range(NKS):
                    nc.tensor.matmul(
                        qp[:], wq_sb[:, ks, mb * P:(mb + 1) * P],
                        xt_ch[:, ks, :],
                        start=(ks == 0), stop=(ks == NKS - 1))
                nc.scalar.activation(
                    qt_sb[:, mb, c * CH:(c + 1) * CH], qp[:], AF.Copy, scale=SCALE)
            # k^T chunk
            kp = proj_ps.tile([P, CH], F32, tag="proj")
            for ks in range(NKS):
                nc.tensor.matmul(kp[:], wk_sb[:, ks, :],
                                 xt_ch[:, ks, :],
                                 start=(ks == 0), stop=(ks == NKS - 1))
            nc.vector.tensor_copy(kt_sb[:, c * CH:(c + 1) * CH], kp[:])
            # v^T chunk, then PE-transpose into vones (v in natural [t, dh] layout)
            vp = proj_ps.tile([P, CH], F32, tag="proj")
            for ks in range(NKS):
                nc.tensor.matmul(vp[:], wv_sb[:, ks, :],
                                 xt_ch[:, ks, :],
                                 start=(ks == 0), stop=(ks == NKS - 1))
            vt_sb = row_p.tile([P, CH], F32, tag="vt")
            nc.vector.tensor_copy(vt_sb[:], vp[:])
            for r in range(CH // P):
                ki = c * (CH // P) + r
                tp = aux_ps.tile([P, P], F32, tag="aux")
                nc.tensor.transpose(tp[:], vt_sb[:, r * P:(r + 1) * P], identity)
                for j in range(NKV):
                    nc.vector.tensor_copy(vones[:, j, ki, 0:DH],
                                          tp[:, j * DH:(j + 1) * DH])

        # prefetch Wo (slot freed by wq after phase A)
        wo_sb = big.tile([P, DQ // P, D], F32R, tag="big")
        nc.sync.dma_start(wo_sb[:], wo_v.bitcast(F32R))

        # ---- Phases B+C interleaved per T_q block ----
        # q heads are permuted host-side to order [0,4,1,5,2,6,3,7] so that
        # head h sits at (block h%4, partition offset 64*(h//4)) -- the
        # partition offset then always equals its kv head's offset in kt_sb,
        # satisfying matmul's equal-base-partition requirement.
        # out^T is kept per-T_q-block so the Wo matmuls for block tb can
        # overlap the (ACT-bound) attention of block tb+1.
        for tb in range(NTQB):
            outt_tb = ot_p.tile([P, DQ // P, TQB], F32R, tag="ot")
            for h in range(NHQ):
                j = h // GROUP            # kv head on this core
                mbq, poq = h % 4, (h // GROUP) * DH
                pv = pv_ps.tile([DH + 1, TQB], F32, tag="pv")
                for ki in range(NKI):
                    sp = s_ps.tile([P, TQB], F32, tag="s")
                    nc.tensor.matmul(
                        sp[:], kt_sb[j * DH:(j + 1) * DH, ki * P:(ki + 1) * P],
                        qt_sb[poq:poq + DH, mbq, tb * TQB:(tb + 1) * TQB],
                        start=True, stop=True)
                    ex = exp_p.tile([P, TQB], F32R, tag="exp")
                    nc.scalar.activation(ex[:], sp[:], AF.Exp)
                    nc.tensor.matmul(pv[:], vones[:, j, ki, :],
                                     ex[:],
                                     start=(ki == 0), stop=(ki == NKI - 1))
                rc = rc_p.tile([1, TQB], F32, tag="rc")
                nc.vector.reciprocal(rc[:], pv[DH:DH + 1, :])
                bc = bc_p.tile([DH, TQB], F32, tag="bc")
                nc.gpsimd.partition_broadcast(bc[:], rc[:], channels=DH)
                nc.vector.tensor_mul(
                    outt_tb[poq:poq + DH, mbq, :],
                    pv[0:DH, :], bc[:])
            # Wo for the 4 output row-tiles covered by this block
            for mi in range(TQB // P):
                mt = tb * (TQB // P) + mi
                y_sb = row_p.tile([P, D], F32, tag="rows")
                for nb in range(NB):
                    yp = proj_ps.tile([P, 512], F32, tag="proj")
                    for ks in range(DQ // P):
                        nc.tensor.matmul(
                            yp[:], outt_tb[:, ks, mi * P:(mi + 1) * P],
                            wo_sb[:, ks, nb * 512:(nb + 1) * 512],
                            start=(ks == 0), stop=(ks == DQ // P - 1))
                    nc.vector.tensor_copy(y_sb[:, nb * 512:(nb + 1) * 512], yp[:])
                nc.sync.dma_start(y_v[:, mt, :], y_sb[:])

    nc.compile()
    return nc


_NC_CACHE = {}


def _get_nc():
    if "nc" not in _NC_CACHE:
        _NC_CACHE["nc"] = _build()
    return _NC_CACHE["nc"]


def make_in_maps(inputs):
    x = np.ascontiguousarray(np.asarray(inputs["x"], dtype=np.float32))
    Wq = np.asarray(inputs["Wq"], dtype=np.float32)
    Wk = np.asarray(inputs["Wk"], dtype=np.float32)
    Wv = np.asarray(inputs["Wv"], dtype=np.float32)
    Wo = np.asarray(inputs["Wo"], dtype=np.float32)
    # interleave the per-core q heads as [0,4,1,5,2,6,3,7] (see phase B note)
    perm = np.concatenate(
        [np.r_[b * DH:(b + 1) * DH, (b + 4) * DH:(b + 5) * DH] for b in range(4)])
    in_maps = []
    for c in range(NCORES):
        b, g = divmod(c, TP)
        in_maps.append({
            "x": x[b],
            "wq": np.ascontiguousarray(Wq[:, g * DQ:(g + 1) * DQ][:, perm]),
            "wk": np.ascontiguousarray(Wk[:, g * DKV:(g + 1) * DKV]),
            "wv": np.ascontiguousarray(Wv[:, g * DKV:(g + 1) * DKV]),
            "wo": np.ascontiguousarray(Wo[g * DQ:(g + 1) * DQ, :][perm, :]),
        })
    return in_maps


def kernel(x, Wq, Wk, Wv, Wo):
    nc = _get_nc()
    in_maps = make_in_maps({"x": x, "Wq": Wq, "Wk": Wk, "Wv": Wv, "Wo": Wo})
    res = run_bass_kernel_spmd(nc, in_maps, list(range(NCORES)))
    y = np.zeros((B, T, D), dtype=np.float32)
    for c in range(NCORES):
        b = c // TP
        y[b] += res.results[c]["y"]
    return y



# revision 7
# speedup vs baseline: 1.5970x; 1.5970x over previous
"""GQA attention block on 8 NeuronCores.

Sharding: tensor-parallel over head groups (4 ways: 8 q heads / 2 kv heads
per core) x data-parallel over batch (2 ways).  Each core computes a partial
y = attn_out_slice @ Wo_slice for its (batch, head-group); the host sums the
4 TP partials per batch element.

All matmul operands are bf16 (host-converted): fp32r matmuls at sustained
full rate trip the power throttle (~50% util limit for most of the run in
the v0 trace); bf16 streams at the same 1 col/cycle without throttling and
halves DMA + weight-load traffic.  PSUM accumulation stays fp32.

Per-core device program:
  A) x^T via PE transposes (grouped 4-per-PSUM-bank, single strided
     eviction copy alternating DVE/ACT); k^T/v^T projections per chunk.
  B) per T_q block: q^T projection for that chunk, then attention:
     S^T tiles = k^T.T @ q^T (scale folded into Wq host-side), exp on ACT
     batched over 2 PSUM banks, PV via lhsT=[v|ones] (row 64 = softmax
     denominator), normalize with reciprocal_approx_fast + partition
     broadcast + multiply.  Wo matmuls for block tb overlap block tb+1.
"""

import os
import sys

import numpy as np

for _p in ("/opt/trn_rl_repo",):
    if os.path.isdir(_p) and _p not in sys.path:
        sys.path.insert(0, _p)

from contextlib import ExitStack

import ml_dtypes

import concourse.bass as bass  # noqa: F401  (AP types pulled in transitively)
import concourse.mybir as mybir
import concourse.tile as tile
from concourse import bacc
from concourse.bass_utils import run_bass_kernel_spmd
from concourse.masks import make_identity

P = 128
B, T, D = 2, 2048, 2048
HQ, HKV, DH = 32, 8, 64
GROUP = HQ // HKV            # 4
TP = 4                       # tensor-parallel ways
DP = 2                       # data-parallel ways
NCORES = TP * DP
DQ = D // TP                 # 512 q dims per core (8 heads)
DKV = HKV * DH // TP         # 128 kv dims per core (2 kv heads)
NHQ = HQ // TP               # 8 q heads per core
NKV = HKV // TP              # 2 kv heads per core
NKS = D // P                 # 16 contraction subtiles over D
CH = 512                     # T chunk width in projection phase
NCH = T // CH                # 4
TQB = 512                    # T_q block width in attention / psum bank
NTQB = T // TQB              # 4
NKI = T // P                 # 16 key tiles
EG = 2                       # ki tiles per exp group (PSUM banks per S tile)
NB = D // 512                # 4 output column banks
SCALE = 1.0 / 8.0            # 1/sqrt(DH), folded into Wq on the host
F32 = mybir.dt.float32
BF16 = mybir.dt.bfloat16
AF = mybir.ActivationFunctionType
NPBF16 = np.dtype(ml_dtypes.bfloat16)


def _build():
    nc = bacc.Bacc(None, target_bir_lowering=False, debug=False)

    x_ext = nc.dram_tensor("x", [T, D], BF16, kind="ExternalInput")
    wq_ext = nc.dram_tensor("wq", [D, DQ], BF16, kind="ExternalInput")
    wk_ext = nc.dram_tensor("wk", [D, DKV], BF16, kind="ExternalInput")
    wv_ext = nc.dram_tensor("wv", [D, DKV], BF16, kind="ExternalInput")
    wo_ext = nc.dram_tensor("wo", [DQ, D], BF16, kind="ExternalInput")
    y_ext = nc.dram_tensor("y", [T, D], F32, kind="ExternalOutput")

    x_v = x_ext[:].rearrange("(to p) d -> p to d", p=P)      # [128,16,2048]
    wq_v = wq_ext[:].rearrange("(ko p) m -> p ko m", p=P)    # [128,16,512]
    wk_v = wk_ext[:].rearrange("(ko p) m -> p ko m", p=P)    # [128,16,128]
    wv_v = wv_ext[:].rearrange("(ko p) m -> p ko m", p=P)
    wo_v = wo_ext[:].rearrange("(ko p) n -> p ko n", p=P)    # [128,4,2048]
    y_v = y_ext[:].rearrange("(to p) n -> p to n", p=P)      # [128,16,2048]

    with tile.TileContext(nc) as tc, ExitStack() as ctx:
        const = ctx.enter_context(tc.tile_pool(name="const", bufs=1))
        w_p = ctx.enter_context(tc.tile_pool(name="wp", bufs=1))
        xt_p = ctx.enter_context(tc.tile_pool(name="xtp", bufs=1))
        row_p = ctx.enter_context(tc.tile_pool(name="rows", bufs=3))
        vt_p = ctx.enter_context(tc.tile_pool(name="vtp", bufs=2))
        qt_p = ctx.enter_context(tc.tile_pool(name="qt", bufs=1))
        kt_p = ctx.enter_context(tc.tile_pool(name="kt", bufs=1))
        vo_p = ctx.enter_context(tc.tile_pool(name="vo", bufs=1))
        exp_p = ctx.enter_context(tc.tile_pool(name="expp", bufs=2))
        bc_p = ctx.enter_context(tc.tile_pool(name="bcp", bufs=2))
        rc_p = ctx.enter_context(tc.tile_pool(name="rcp", bufs=2))
        ot_p = ctx.enter_context(tc.tile_pool(name="otp", bufs=2))
        y_p = ctx.enter_context(tc.tile_pool(name="yp", bufs=2))

        proj_ps = ctx.enter_context(tc.tile_pool(name="proj_ps", bufs=2, space="PSUM"))

        identity = const.tile([P, P], BF16)
        make_identity(nc, identity)

        wq_sb = w_p.tile([P, NKS, DQ], BF16)
        wk_sb = w_p.tile([P, NKS, DKV], BF16)
        wv_sb = w_p.tile([P, NKS, DKV], BF16)
        wo_sb = w_p.tile([P, DQ // P, D], BF16)

        xt_sb = xt_p.tile([P, NKS, T], BF16)            # x^T, [d%128, d//128, t]
        qt_sb = qt_p.tile([P, DQ // P, T], BF16)        # q^T * SCALE, [dim, t]
        kt_sb = kt_p.tile([P, T], BF16)                 # k^T, [dim(2 kv heads), t]
        vones = vo_p.tile([P, NKV, NKI, DH + 1], BF16)  # [t%128, kv, t//128, dh|1]
        ones_col = const.tile([P, NKV, NKI], BF16)
        nc.gpsimd.memset(ones_col[:], 1.0)
        nc.vector.tensor_copy(vones[:, :, :, DH], ones_col[:])

        # ---- Phase A: x^T + k^T/v^T projections, chunk by chunk ----
        with tc.tile_pool(name="tp_ps", bufs=2, space="PSUM") as tp_ps:
            copy_flip = 0
            for c in range(NCH):
                xrows = []
                for r in range(CH // P):
                    xrow = row_p.tile([P, D], BF16, tag="rows")
                    nc.sync.dma_start(xrow[:], x_v[:, c * (CH // P) + r, :])
                    xrows.append(xrow)
                if c == 0:
                    # weights go out after the first x rows so PE transposes
                    # start as early as possible
                    nc.sync.dma_start(wq_sb[:], wq_v)
                    nc.sync.dma_start(wk_sb[:], wk_v)
                    nc.sync.dma_start(wv_sb[:], wv_v)
                for r in range(CH // P):
                    t0 = (c * (CH // P) + r) * P
                    for g in range(NKS // 4):
                        # 4 transposes share one PSUM bank: they must form ONE
                        # accumulation group (start only on the first) -- each
                        # start_tensor_calc zeroes the whole 2KB zero region on
                        # hardware, wiping earlier writes in the bank.
                        tp = tp_ps.tile([P, 4, P], BF16, tag="tp")
                        for i in range(4):
                            nc.tensor.matmul(
                                tp[:, i, :],
                                xrows[r][:, (g * 4 + i) * P:(g * 4 + i + 1) * P],
                                identity, is_transpose=True,
                                start=(i == 0), stop=(i == 3))
                        dst = xt_sb[:, g * 4:(g + 1) * 4, t0:t0 + P]
                        if copy_flip & 1:
                            nc.vector.tensor_copy(dst, tp[:])
                        else:
                            nc.scalar.activation(dst, tp[:], AF.Copy)
                        copy_flip += 1
                # k^T chunk
                kp = proj_ps.tile([P, CH], F32, tag="proj")
                for ks in range(NKS):
                    nc.tensor.matmul(kp[:], wk_sb[:, ks, :],
                                     xt_sb[:, ks, c * CH:(c + 1) * CH],
                                     start=(ks == 0), stop=(ks == NKS - 1))
                nc.scalar.activation(kt_sb[:, c * CH:(c + 1) * CH], kp[:], AF.Copy)
                # v^T chunk, then PE-transpose into vones (v in natural [t, dh])
                vp = proj_ps.tile([P, CH], F32, tag="proj")
                for ks in range(NKS):
                    nc.tensor.matmul(vp[:], wv_sb[:, ks, :],
                                     xt_sb[:, ks, c * CH:(c + 1) * CH],
                                     start=(ks == 0), stop=(ks == NKS - 1))
                vt_sb = vt_p.tile([P, CH], BF16, tag="vt")
                nc.scalar.activation(vt_sb[:], vp[:], AF.Copy)
                tpv = tp_ps.tile([P, 4, P], BF16, tag="tp")
                for r in range(CH // P):
                    nc.tensor.matmul(tpv[:, r, :],
                                     vt_sb[:, r * P:(r + 1) * P],
                                     identity, is_transpose=True,
                                     start=(r == 0), stop=(r == 3))
                for r in range(CH // P):
                    ki = c * (CH // P) + r
                    for j in range(NKV):
                        nc.scalar.activation(vones[:, j, ki, 0:DH],
                                             tpv[:, r, j * DH:(j + 1) * DH],
                                             AF.Copy)

        # prefetch Wo
        nc.sync.dma_start(wo_sb[:], wo_v)

        s_ps = ctx.enter_context(tc.tile_pool(name="s_ps", bufs=2, space="PSUM"))
        pv_ps = ctx.enter_context(tc.tile_pool(name="pv_ps", bufs=2, space="PSUM"))

        # ---- Phase B: q^T chunk c, then attention T_q block c ----
        # q heads are permuted host-side to order [0,4,1,5,2,6,3,7] so that
        # head h sits at (block h%4, partition offset 64*(h//4)) -- the
        # partition offset then always equals its kv head's offset in kt_sb,
        # satisfying matmul's equal-base-partition requirement.
        for tb in range(NTQB):
            # q^T for this block (scale pre-folded into Wq)
            for mb in range(DQ // P):
                qp = proj_ps.tile([P, CH], F32, tag="proj")
                for ks in range(NKS):
                    nc.tensor.matmul(
                        qp[:], wq_sb[:, ks, mb * P:(mb + 1) * P],
                        xt_sb[:, ks, tb * CH:(tb + 1) * CH],
                        start=(ks == 0), stop=(ks == NKS - 1))
                nc.vector.tensor_copy(qt_sb[:, mb, tb * CH:(tb + 1) * CH], qp[:])

            outt_tb = ot_p.tile([P, DQ // P, TQB], BF16, tag="ot")
            for h in range(NHQ):
                j = h // GROUP            # kv head on this core
                mbq, poq = h % 4, (h // GROUP) * DH
                pv = pv_ps.tile([DH + 1, TQB], F32, tag="pv")
                for g in range(NKI // EG):
                    sp = s_ps.tile([P, EG, TQB], F32, tag="s")
                    for i in range(EG):
                        ki = g * EG + i
                        nc.tensor.matmul(
                            sp[:, i, :],
                            kt_sb[j * DH:(j + 1) * DH, ki * P:(ki + 1) * P],
                            qt_sb[poq:poq + DH, mbq, tb * TQB:(tb + 1) * TQB],
                            start=True, stop=True)
                    ex = exp_p.tile([P, EG, TQB], BF16, tag="exp")
                    nc.scalar.activation(ex[:], sp[:], AF.Exp)
                    for i in range(EG):
                        ki = g * EG + i
                        nc.tensor.matmul(pv[:], vones[:, j, ki, :],
                                         ex[:, i, :],
                                         start=(ki == 0), stop=(ki == NKI - 1))
                # reciprocal_approx_fast mislowers a non-zero input partition
                # base on HW -- stage the denominator row to partition 0 first
                den = rc_p.tile([1, TQB], F32, tag="den")
                nc.vector.tensor_copy(den[:], pv[DH:DH + 1, :])
                rc = rc_p.tile([1, TQB], F32, tag="rc")
                nc.vector.reciprocal_approx_fast(rc[:], den[:])
                bc = bc_p.tile([DH, TQB], F32, tag="bc")
                nc.gpsimd.partition_broadcast(bc[:], rc[:], channels=DH)
                nc.vector.tensor_mul(
                    outt_tb[poq:poq + DH, mbq, :],
                    pv[0:DH, :], bc[:])
            # Wo for the 4 output row-tiles covered by this block
            for mi in range(TQB // P):
                mt = tb * (TQB // P) + mi
                y_sb = y_p.tile([P, D], F32, tag="y")
                for nb in range(NB):
                    yp = proj_ps.tile([P, 512], F32, tag="proj")
                    for ks in range(DQ // P):
                        nc.tensor.matmul(
                            yp[:], outt_tb[:, ks, mi * P:(mi + 1) * P],
                            wo_sb[:, ks, nb * 512:(nb + 1) * 512],
                            start=(ks == 0), stop=(ks == DQ // P - 1))
                    nc.vector.tensor_copy(y_sb[:, nb * 512:(nb + 1) * 512], yp[:])
                nc.sync.dma_start(y_v[:, mt, :], y_sb[:])

    nc.compile()
    return nc


_NC_CACHE = {}


def _get_nc():
    if "nc" not in _NC_CACHE:
        _NC_CACHE["nc"] = _build()
    return _NC_CACHE["nc"]


def make_in_maps(inputs):
    x = np.asarray(inputs["x"], dtype=np.float32)
    Wq = np.asarray(inputs["Wq"], dtype=np.float32) * SCALE
    Wk = np.asarray(inputs["Wk"], dtype=np.float32)
    Wv = np.asarray(inputs["Wv"], dtype=np.float32)
    Wo = np.asarray(inputs["Wo"], dtype=np.float32)
    # interleave the per-core q heads as [0,4,1,5,2,6,3,7] (see phase B note)
    perm = np.concatenate(
        [np.r_[b * DH:(b + 1) * DH, (b + 4) * DH:(b + 5) * DH] for b in range(4)])
    xb = [np.ascontiguousarray(x[b].astype(NPBF16)) for b in range(B)]
    in_maps = []
    for c in range(NCORES):
        b, g = divmod(c, TP)
        in_maps.append({
            "x": xb[b],
            "wq": np.ascontiguousarray(
                Wq[:, g * DQ:(g + 1) * DQ][:, perm].astype(NPBF16)),
            "wk": np.ascontiguousarray(
                Wk[:, g * DKV:(g + 1) * DKV].astype(NPBF16)),
            "wv": np.ascontiguousarray(
                Wv[:, g * DKV:(g + 1) * DKV].astype(NPBF16)),
            "wo": np.ascontiguousarray(
                Wo[g * DQ:(g + 1) * DQ, :][perm, :].astype(NPBF16)),
        })
    return in_maps


def kernel(x, Wq, Wk, Wv, Wo):
    nc = _get_nc()
    in_maps = make_in_maps({"x": x, "Wq": Wq, "Wk": Wk, "Wv": Wv, "Wo": Wo})
    res = run_bass_kernel_spmd(nc, in_maps, list(range(NCORES)))
    y = np.zeros((B, T, D), dtype=np.float32)
    for c in range(NCORES):
        b = c // TP
        y[b] += res.results[c]["y"]
    return y


# revision 11
# speedup vs baseline: 1.6210x; 1.0150x over previous
"""GQA attention block on 8 NeuronCores.

Sharding: tensor-parallel over head groups (4 ways: 8 q heads / 2 kv heads
per core) x data-parallel over batch (2 ways).  Each core computes a partial
y = attn_out_slice @ Wo_slice for its (batch, head-group); the host sums the
4 TP partials per batch element.

All matmul operands are bf16 (host-converted): fp32r matmuls at sustained
full rate trip the power throttle (~50% util limit for most of the run in
the v0 trace); bf16 streams at the same 1 col/cycle without throttling and
halves DMA + weight-load traffic.  PSUM accumulation stays fp32.

Per-core device program:
  A) x^T via PE transposes (grouped 4-per-PSUM-bank, single strided
     eviction copy alternating DVE/ACT); k^T/v^T projections per chunk.
  B) per T_q block: q^T projection for that chunk, then attention:
     S^T tiles = k^T.T @ q^T (scale folded into Wq host-side), exp on ACT
     batched over 2 PSUM banks, PV via lhsT=[v|ones] (row 64 = softmax
     denominator), normalize with reciprocal_approx_fast + partition
     broadcast + multiply.  Wo matmuls for block tb overlap block tb+1.
"""

import os
import sys

import numpy as np

for _p in ("/opt/trn_rl_repo",):
    if os.path.isdir(_p) and _p not in sys.path:
        sys.path.insert(0, _p)

from contextlib import ExitStack

import ml_dtypes

import concourse.bass as bass  # noqa: F401  (AP types pulled in transitively)
import concourse.mybir as mybir
import concourse.tile as tile
from concourse import bacc
from concourse.bass_utils import run_bass_kernel_spmd
from concourse.masks import make_identity

P = 128
B, T, D = 2, 2048, 2048
HQ, HKV, DH = 32, 8, 64
GROUP = HQ // HKV            # 4
TP = 4                       # tensor-parallel ways
DP = 2                       # data-parallel ways
NCORES = TP * DP
DQ = D // TP                 # 512 q dims per core (8 heads)
DKV = HKV * DH // TP         # 128 kv dims per core (2 kv heads)
NHQ = HQ // TP               # 8 q heads per core
NKV = HKV // TP              # 2 kv heads per core
NKS = D // P                 # 16 contraction subtiles over D
CH = 512                     # T chunk width in projection phase
NCH = T // CH                # 4
TQB = 512                    # T_q block width in attention / psum bank
NTQB = T // TQB              # 4
NKI = T // P                 # 16 key tiles
EG = 2                       # ki tiles per exp group (PSUM banks per S tile)
NB = D // 512                # 4 output column banks
SCALE = 1.0 / 8.0            # 1/sqrt(DH), folded into Wq on the host
F32 = mybir.dt.float32
BF16 = mybir.dt.bfloat16
AF = mybir.ActivationFunctionType
NPBF16 = np.dtype(ml_dtypes.bfloat16)


def _build():
    nc = bacc.Bacc(None, target_bir_lowering=False, debug=False)

    x_ext = nc.dram_tensor("x", [T, D], BF16, kind="ExternalInput")
    wq_ext = nc.dram_tensor("wq", [D, DQ], BF16, kind="ExternalInput")
    wk_ext = nc.dram_tensor("wk", [D, DKV], BF16, kind="ExternalInput")
    wv_ext = nc.dram_tensor("wv", [D, DKV], BF16, kind="ExternalInput")
    wo_ext = nc.dram_tensor("wo", [DQ, D], BF16, kind="ExternalInput")
    y_ext = nc.dram_tensor("y", [T, D], F32, kind="ExternalOutput")

    x_v = x_ext[:].rearrange("(to p) d -> p to d", p=P)      # [128,16,2048]
    wq_v = wq_ext[:].rearrange("(ko p) m -> p ko m", p=P)    # [128,16,512]
    wk_v = wk_ext[:].rearrange("(ko p) m -> p ko m", p=P)    # [128,16,128]
    wv_v = wv_ext[:].rearrange("(ko p) m -> p ko m", p=P)
    wo_v = wo_ext[:].rearrange("(ko p) n -> p ko n", p=P)    # [128,4,2048]
    y_v = y_ext[:].rearrange("(to p) n -> p to n", p=P)      # [128,16,2048]

    with tile.TileContext(nc) as tc, ExitStack() as ctx:
        const = ctx.enter_context(tc.tile_pool(name="const", bufs=1))
        w_p = ctx.enter_context(tc.tile_pool(name="wp", bufs=1))
        xt_p = ctx.enter_context(tc.tile_pool(name="xtp", bufs=1))
        row_p = ctx.enter_context(tc.tile_pool(name="rows", bufs=3))
        vt_p = ctx.enter_context(tc.tile_pool(name="vtp", bufs=2))
        qt_p = ctx.enter_context(tc.tile_pool(name="qt", bufs=1))
        kt_p = ctx.enter_context(tc.tile_pool(name="kt", bufs=1))
        vo_p = ctx.enter_context(tc.tile_pool(name="vo", bufs=1))
        exp_p = ctx.enter_context(tc.tile_pool(name="expp", bufs=3))
        bc_p = ctx.enter_context(tc.tile_pool(name="bcp", bufs=2))
        rc_p = ctx.enter_context(tc.tile_pool(name="rcp", bufs=2))
        ot_p = ctx.enter_context(tc.tile_pool(name="otp", bufs=2))
        y_p = ctx.enter_context(tc.tile_pool(name="yp", bufs=2))

        identity = const.tile([P, P], BF16)
        make_identity(nc, identity)

        wq_sb = w_p.tile([P, NKS, DQ], BF16)
        wk_sb = w_p.tile([P, NKS, DKV], BF16)
        wv_sb = w_p.tile([P, NKS, DKV], BF16)
        wo_sb = w_p.tile([P, DQ // P, D], BF16)

        xt_sb = xt_p.tile([P, NKS, T], BF16)            # x^T, [d%128, d//128, t]
        qt_sb = qt_p.tile([P, DQ // P, T], BF16)        # q^T * SCALE, [dim, t]
        kt_sb = kt_p.tile([P, T], BF16)                 # k^T, [dim(2 kv heads), t]
        vones = vo_p.tile([P, NKV, NKI, DH + 1], BF16)  # [t%128, kv, t//128, dh|1]
        ones_col = const.tile([P, NKV, NKI], BF16)
        nc.gpsimd.memset(ones_col[:], 1.0)
        nc.vector.tensor_copy(vones[:, :, :, DH], ones_col[:])

        # ---- Phase A: x^T + k^T/v^T projections, chunk by chunk ----
        with tc.tile_pool(name="tp_ps", bufs=2, space="PSUM") as tp_ps, \
             tc.tile_pool(name="proj_ps", bufs=2, space="PSUM") as proj_ps:
            copy_flip = 0
            for c in range(NCH):
                xrows = []
                for r in range(CH // P):
                    xrow = row_p.tile([P, D], BF16, tag="rows")
                    nc.sync.dma_start(xrow[:], x_v[:, c * (CH // P) + r, :])
                    xrows.append(xrow)
                if c == 0:
                    # weights go out after the first x rows so PE transposes
                    # start as early as possible
                    nc.sync.dma_start(wq_sb[:], wq_v)
                    nc.sync.dma_start(wk_sb[:], wk_v)
                    nc.sync.dma_start(wv_sb[:], wv_v)
                for r in range(CH // P):
                    t0 = (c * (CH // P) + r) * P
                    for g in range(NKS // 4):
                        # 4 transposes share one PSUM bank: they must form ONE
                        # accumulation group (start only on the first) -- each
                        # start_tensor_calc zeroes the whole 2KB zero region on
                        # hardware, wiping earlier writes in the bank.
                        tp = tp_ps.tile([P, 4, P], BF16, tag="tp")
                        for i in range(4):
                            nc.tensor.matmul(
                                tp[:, i, :],
                                xrows[r][:, (g * 4 + i) * P:(g * 4 + i + 1) * P],
                                identity, is_transpose=True,
                                start=(i == 0), stop=(i == 3))
                        dst = xt_sb[:, g * 4:(g + 1) * 4, t0:t0 + P]
                        if copy_flip & 1:
                            nc.vector.tensor_copy(dst, tp[:])
                        else:
                            nc.scalar.activation(dst, tp[:], AF.Copy)
                        copy_flip += 1
                # k^T chunk
                kp = proj_ps.tile([P, CH], F32, tag="proj")
                for ks in range(NKS):
                    nc.tensor.matmul(kp[:], wk_sb[:, ks, :],
                                     xt_sb[:, ks, c * CH:(c + 1) * CH],
                                     start=(ks == 0), stop=(ks == NKS - 1))
                nc.scalar.activation(kt_sb[:, c * CH:(c + 1) * CH], kp[:], AF.Copy)
                # v^T chunk, then PE-transpose into vones (v in natural [t, dh])
                vp = proj_ps.tile([P, CH], F32, tag="proj")
                for ks in range(NKS):
                    nc.tensor.matmul(vp[:], wv_sb[:, ks, :],
                                     xt_sb[:, ks, c * CH:(c + 1) * CH],
                                     start=(ks == 0), stop=(ks == NKS - 1))
                vt_sb = vt_p.tile([P, CH], BF16, tag="vt")
                nc.scalar.activation(vt_sb[:], vp[:], AF.Copy)
                tpv = tp_ps.tile([P, 4, P], BF16, tag="tp")
                for r in range(CH // P):
                    nc.tensor.matmul(tpv[:, r, :],
                                     vt_sb[:, r * P:(r + 1) * P],
                                     identity, is_transpose=True,
                                     start=(r == 0), stop=(r == 3))
                for r in range(CH // P):
                    ki = c * (CH // P) + r
                    for j in range(NKV):
                        nc.scalar.activation(vones[:, j, ki, 0:DH],
                                             tpv[:, r, j * DH:(j + 1) * DH],
                                             AF.Copy)

        # prefetch Wo
        nc.sync.dma_start(wo_sb[:], wo_v)

        qproj_ps = ctx.enter_context(
            tc.tile_pool(name="qproj_ps", bufs=1, space="PSUM"))
        wo_ps = ctx.enter_context(tc.tile_pool(name="wo_ps", bufs=1, space="PSUM"))
        s_ps = ctx.enter_context(tc.tile_pool(name="s_ps", bufs=2, space="PSUM"))
        pv_ps = ctx.enter_context(tc.tile_pool(name="pv_ps", bufs=2, space="PSUM"))

        # ---- Phase B: software-pipelined attention ----
        # q heads are permuted host-side to order [0,4,1,5,2,6,3,7] so that
        # head h sits at (block h%4, partition offset 64*(h//4)) -- the
        # partition offset then always equals its kv head's offset in kt_sb,
        # satisfying matmul's equal-base-partition requirement.
        #
        # The PE executes its queue in order, so any instruction that waits
        # stalls everything behind it AND drops the clock out of the ramped
        # p-state.  Three measures keep the PE stream gapless:
        #   - PV matmuls lag their exp group by one full group, covering the
        #     ~1.2us ACT exp latency;
        #   - the Wo matmuls of block tb-1 and the q^T projection of block
        #     tb+1 are interleaved as ACT-independent fillers (2 per group);
        #   - normalization (reciprocal/broadcast/mul) runs on DVE/Pool a
        #     full block ahead of the Wo matmuls that consume its output.

        def qproj_emit(tb):
            """q^T for block tb (scale pre-folded into Wq); yields per matmul."""
            for mb in range(DQ // P):
                qp = qproj_ps.tile([P, CH], F32, tag="qproj")
                for ks in range(NKS):
                    nc.tensor.matmul(
                        qp[:], wq_sb[:, ks, mb * P:(mb + 1) * P],
                        xt_sb[:, ks, tb * CH:(tb + 1) * CH],
                        start=(ks == 0), stop=(ks == NKS - 1))
                    yield
                nc.vector.tensor_copy(qt_sb[:, mb, tb * CH:(tb + 1) * CH], qp[:])

        def wo_emit(tb, outt):
            """y = outt.T @ Wo for block tb; yields per matmul."""
            for mi in range(TQB // P):
                y_sb = y_p.tile([P, D], F32, tag="y")
                for nb in range(NB):
                    yp = wo_ps.tile([P, 512], F32, tag="wo")
                    for ks in range(DQ // P):
                        nc.tensor.matmul(
                            yp[:], outt[:, ks, mi * P:(mi + 1) * P],
                            wo_sb[:, ks, nb * 512:(nb + 1) * 512],
                            start=(ks == 0), stop=(ks == DQ // P - 1))
                        yield
                    nc.vector.tensor_copy(y_sb[:, nb * 512:(nb + 1) * 512], yp[:])
                nc.sync.dma_start(y_v[:, tb * (TQB // P) + mi, :], y_sb[:])

        def normalize(pv, outt, h):
            mbq, poq = h % 4, (h // GROUP) * DH
            # reciprocal_approx_fast mislowers a non-zero input partition
            # base on HW -- stage the denominator row to partition 0 first
            den = rc_p.tile([1, TQB], F32, tag="den")
            nc.vector.tensor_copy(den[:], pv[DH:DH + 1, :])
            rc = rc_p.tile([1, TQB], F32, tag="rc")
            nc.vector.reciprocal_approx_fast(rc[:], den[:])
            bc = bc_p.tile([DH, TQB], F32, tag="bc")
            nc.gpsimd.partition_broadcast(bc[:], rc[:], channels=DH)
            nc.vector.tensor_mul(outt[poq:poq + DH, mbq, :], pv[0:DH, :], bc[:])

        # q^T for block 0 runs unpipelined ahead of the first attention block
        for _ in qproj_emit(0):
            pass

        NG = NKI // EG               # exp groups per head
        prev_outt = None
        for tb in range(NTQB):
            fillers = []
            if prev_outt is not None:
                fillers.append(wo_emit(tb - 1, prev_outt))
            if tb + 1 < NTQB:
                fillers.append(qproj_emit(tb + 1))
            filler = (f for gen in fillers for f in gen)

            outt_tb = ot_p.tile([P, DQ // P, TQB], BF16, tag="ot")
            lag = []                 # deferred PV groups: (pv, ex, j, g)
            pv = None
            for G in range(NHQ * NG):
                h, g = divmod(G, NG)
                j = h // GROUP       # kv head on this core
                mbq, poq = h % 4, (h // GROUP) * DH
                if g == 0:
                    pv = pv_ps.tile([DH + 1, TQB], F32, tag="pv")
                sp = s_ps.tile([P, EG, TQB], F32, tag="s")
                for i in range(EG):
                    ki = g * EG + i
                    nc.tensor.matmul(
                        sp[:, i, :],
                        kt_sb[j * DH:(j + 1) * DH, ki * P:(ki + 1) * P],
                        qt_sb[poq:poq + DH, mbq, tb * TQB:(tb + 1) * TQB],
                        start=True, stop=True)
                ex = exp_p.tile([P, EG, TQB], BF16, tag="exp")
                nc.scalar.activation(ex[:], sp[:], AF.Exp)
                lag.append((pv, ex, h, g))
                if len(lag) > 1:
                    lpv, lex, lh, lg = lag.pop(0)
                    for i in range(EG):
                        ki = lg * EG + i
                        nc.tensor.matmul(lpv[:], vones[:, lh // GROUP, ki, :],
                                         lex[:, i, :],
                                         start=(ki == 0), stop=(ki == NKI - 1))
                    if lg == NG - 1:
                        normalize(lpv, outt_tb, lh)
                for _ in range(2):
                    next(filler, None)
            # drain: last PV group + its normalize, then remaining fillers
            lpv, lex, lh, lg = lag.pop(0)
            for i in range(EG):
                ki = lg * EG + i
                nc.tensor.matmul(lpv[:], vones[:, lh // GROUP, ki, :], lex[:, i, :],
                                 start=(ki == 0), stop=(ki == NKI - 1))
            normalize(lpv, outt_tb, lh)
            for _ in filler:
                pass
            prev_outt = outt_tb

        # final Wo block runs unpipelined after the last attention block
        for _ in wo_emit(NTQB - 1, prev_outt):
            pass

    nc.compile()
    return nc


_NC_CACHE = {}


def _get_nc():
    if "nc" not in _NC_CACHE:
        _NC_CACHE["nc"] = _build()
    return _NC_CACHE["nc"]


def make_in_maps(inputs):
    x = np.asarray(inputs["x"], dtype=np.float32)
    Wq = np.asarray(inputs["Wq"], dtype=np.float32) * SCALE
    Wk = np.asarray(inputs["Wk"], dtype=np.float32)
    Wv = np.asarray(inputs["Wv"], dtype=np.float32)
    Wo = np.asarray(inputs["Wo"], dtype=np.float32)
    # interleave the per-core q heads as [0,4,1,5,2,6,3,7] (see phase B note)
    perm = np.concatenate(
        [np.r_[b * DH:(b + 1) * DH, (b + 4) * DH:(b + 5) * DH] for b in range(4)])
    xb = [np.ascontiguousarray(x[b].astype(NPBF16)) for b in range(B)]
    in_maps = []
    for c in range(NCORES):
        b, g = divmod(c, TP)
        in_maps.append({
            "x": xb[b],
            "wq": np.ascontiguousarray(
                Wq[:, g * DQ:(g + 1) * DQ][:, perm].astype(NPBF16)),
            "wk": np.ascontiguousarray(
                Wk[:, g * DKV:(g + 1) * DKV].astype(NPBF16)),
            "wv": np.ascontiguousarray(
                Wv[:, g * DKV:(g + 1) * DKV].astype(NPBF16)),
            "wo": np.ascontiguousarray(
                Wo[g * DQ:(g + 1) * DQ, :][perm, :].astype(NPBF16)),
        })
    return in_maps


def kernel(x, Wq, Wk, Wv, Wo):
    nc = _get_nc()
    in_maps = make_in_maps({"x": x, "Wq": Wq, "Wk": Wk, "Wv": Wv, "Wo": Wo})
    res = run_bass_kernel_spmd(nc, in_maps, list(range(NCORES)))
    y = np.zeros((B, T, D), dtype=np.float32)
    for c in range(NCORES):
        b = c // TP
        y[b] += res.results[c]["y"]
    return y


# revision 16
# speedup vs baseline: 1.7926x; 1.1059x over previous
"""GQA attention block on 8 NeuronCores.

Sharding: tensor-parallel over head groups (4 ways: 8 q heads / 2 kv heads
per core) x data-parallel over batch (2 ways).  Each core computes a partial
y = attn_out_slice @ Wo_slice for its (batch, head-group); the host sums the
4 TP partials per batch element.

All matmul operands are bf16 (host-converted): fp32r matmuls at sustained
full rate trip the power throttle (~50% util limit for most of the run in
the v0 trace); bf16 streams at the same 1 col/cycle without throttling and
halves DMA + weight-load traffic.  PSUM accumulation stays fp32.

Per-core device program:
  A) x^T via PE transposes (grouped 4-per-PSUM-bank, single strided
     eviction copy alternating DVE/ACT); k^T/v^T projections per chunk.
  B) per T_q block: q^T projection for that chunk, then attention:
     S^T tiles = k^T.T @ q^T (scale folded into Wq host-side), exp on ACT
     batched over 2 PSUM banks, PV via lhsT=[v|ones] (row 64 = softmax
     denominator), normalize with reciprocal_approx_fast + partition
     broadcast + multiply.  Wo matmuls for block tb overlap block tb+1.
"""

import os
import sys

import numpy as np

for _p in ("/opt/trn_rl_repo",):
    if os.path.isdir(_p) and _p not in sys.path:
        sys.path.insert(0, _p)

from contextlib import ExitStack

import ml_dtypes

import concourse.bass as bass  # noqa: F401  (AP types pulled in transitively)
import concourse.mybir as mybir
import concourse.tile as tile
from concourse import bacc
from concourse.bass_utils import run_bass_kernel_spmd
from concourse.masks import make_identity

P = 128
B, T, D = 2, 2048, 2048
HQ, HKV, DH = 32, 8, 64
GROUP = HQ // HKV            # 4
TP = 4                       # tensor-parallel ways
DP = 2                       # data-parallel ways
NCORES = TP * DP
DQ = D // TP                 # 512 q dims per core (8 heads)
DKV = HKV * DH // TP         # 128 kv dims per core (2 kv heads)
NHQ = HQ // TP               # 8 q heads per core
NKV = HKV // TP              # 2 kv heads per core
NKS = D // P                 # 16 contraction subtiles over D
CH = 512                     # T chunk width in projection phase
NCH = T // CH                # 4
TQB = 512                    # T_q block width in attention / psum bank
NTQB = T // TQB              # 4
NKI = T // P                 # 16 key tiles
EG = 2                       # ki tiles per exp group (PSUM banks per S tile)
NB = D // 512                # 4 output column banks
SCALE = 1.0 / 8.0            # 1/sqrt(DH), folded into Wq on the host
F32 = mybir.dt.float32
BF16 = mybir.dt.bfloat16
AF = mybir.ActivationFunctionType
NPBF16 = np.dtype(ml_dtypes.bfloat16)


def _build():
    nc = bacc.Bacc(None, target_bir_lowering=False, debug=False)

    x_ext = nc.dram_tensor("x", [T, D], BF16, kind="ExternalInput")
    wq_ext = nc.dram_tensor("wq", [D, DQ], BF16, kind="ExternalInput")
    wk_ext = nc.dram_tensor("wk", [D, DKV], BF16, kind="ExternalInput")
    wv_ext = nc.dram_tensor("wv", [D, DKV], BF16, kind="ExternalInput")
    wo_ext = nc.dram_tensor("wo", [DQ, D], BF16, kind="ExternalInput")
    y_ext = nc.dram_tensor("y", [T, D], F32, kind="ExternalOutput")

    x_v = x_ext[:].rearrange("(to p) d -> p to d", p=P)      # [128,16,2048]
    wq_v = wq_ext[:].rearrange("(ko p) m -> p ko m", p=P)    # [128,16,512]
    wk_v = wk_ext[:].rearrange("(ko p) m -> p ko m", p=P)    # [128,16,128]
    wv_v = wv_ext[:].rearrange("(ko p) m -> p ko m", p=P)
    wo_v = wo_ext[:].rearrange("(ko p) n -> p ko n", p=P)    # [128,4,2048]
    y_v = y_ext[:].rearrange("(to p) n -> p to n", p=P)      # [128,16,2048]

    with tile.TileContext(nc) as tc, ExitStack() as ctx:
        const = ctx.enter_context(tc.tile_pool(name="const", bufs=1))
        w_p = ctx.enter_context(tc.tile_pool(name="wp", bufs=1))
        xt_p = ctx.enter_context(tc.tile_pool(name="xtp", bufs=1))
        row_p = ctx.enter_context(tc.tile_pool(name="rows", bufs=6))
        vt_p = ctx.enter_context(tc.tile_pool(name="vtp", bufs=2))
        qt_p = ctx.enter_context(tc.tile_pool(name="qt", bufs=1))
        kt_p = ctx.enter_context(tc.tile_pool(name="kt", bufs=1))
        vo_p = ctx.enter_context(tc.tile_pool(name="vo", bufs=1))
        exp_p = ctx.enter_context(tc.tile_pool(name="expp", bufs=3))
        bc_p = ctx.enter_context(tc.tile_pool(name="bcp", bufs=2))
        rc_p = ctx.enter_context(tc.tile_pool(name="rcp", bufs=2))
        ot_p = ctx.enter_context(tc.tile_pool(name="otp", bufs=2))
        y_p = ctx.enter_context(tc.tile_pool(name="yp", bufs=2))

        identity = const.tile([P, P], BF16)
        make_identity(nc, identity)

        wq_sb = w_p.tile([P, NKS, DQ], BF16)
        wk_sb = w_p.tile([P, NKS, DKV], BF16)
        wv_sb = w_p.tile([P, NKS, DKV], BF16)
        wo_sb = w_p.tile([P, DQ // P, D], BF16)

        xt_sb = xt_p.tile([P, NKS, T], BF16)            # x^T, [d%128, d//128, t]
        qt_sb = qt_p.tile([P, DQ // P, T], BF16)        # q^T * SCALE, [dim, t]
        kt_sb = kt_p.tile([P, T], BF16)                 # k^T, [dim(2 kv heads), t]
        vones = vo_p.tile([P, NKV, NKI, DH + 1], BF16)  # [t%128, kv, t//128, dh|1]
        ones_col = const.tile([P, NKV, NKI], BF16)
        nc.gpsimd.memset(ones_col[:], 1.0)
        nc.vector.tensor_copy(vones[:, :, :, DH], ones_col[:])

        # ---- Phase A: x^T + k^T/v^T projections, chunk by chunk ----
        with tc.tile_pool(name="tp_ps", bufs=4, space="PSUM") as tp_ps, \
             tc.tile_pool(name="proj_ps", bufs=2, space="PSUM") as proj_ps:

            def kv_emit(c):
                """k^T/v^T projections + v transpose for chunk c; yields per
                PE instruction so it can interleave with the next chunk's
                (DMA-gated) x transposes."""
                kp = proj_ps.tile([P, CH], F32, tag="proj")
                for ks in range(NKS):
                    nc.tensor.matmul(kp[:], wk_sb[:, ks, :],
                                     xt_sb[:, ks, c * CH:(c + 1) * CH],
                                     start=(ks == 0), stop=(ks == NKS - 1))
                    yield
                nc.scalar.activation(kt_sb[:, c * CH:(c + 1) * CH], kp[:], AF.Copy)
                vp = proj_ps.tile([P, CH], F32, tag="proj")
                for ks in range(NKS):
                    nc.tensor.matmul(vp[:], wv_sb[:, ks, :],
                                     xt_sb[:, ks, c * CH:(c + 1) * CH],
                                     start=(ks == 0), stop=(ks == NKS - 1))
                    yield
                vt_sb = vt_p.tile([P, CH], BF16, tag="vt")
                nc.scalar.activation(vt_sb[:], vp[:], AF.Copy)
                tpv = tp_ps.tile([P, 4, P], BF16, tag="tp")
                for r in range(CH // P):
                    nc.tensor.matmul(tpv[:, r, :],
                                     vt_sb[:, r * P:(r + 1) * P],
                                     identity, is_transpose=True,
                                     start=(r == 0), stop=(r == 3))
                    yield
                for r in range(CH // P):
                    ki = c * (CH // P) + r
                    for j in range(NKV):
                        nc.scalar.activation(vones[:, j, ki, 0:DH],
                                             tpv[:, r, j * DH:(j + 1) * DH],
                                             AF.Copy)

            copy_flip = 0
            kv_fill = iter(())
            for c in range(NCH):
                xrows = []
                for r in range(CH // P):
                    xrow = row_p.tile([P, D], BF16, tag="rows")
                    nc.sync.dma_start(xrow[:], x_v[:, c * (CH // P) + r, :])
                    xrows.append(xrow)
                if c == 0:
                    # weights go out after the first x rows so PE transposes
                    # start as early as possible
                    nc.sync.dma_start(wq_sb[:], wq_v)
                    nc.sync.dma_start(wk_sb[:], wk_v)
                    nc.sync.dma_start(wv_sb[:], wv_v)
                for r in range(CH // P):
                    t0 = (c * (CH // P) + r) * P
                    for g in range(NKS // 4):
                        # 4 transposes share one PSUM bank: they must form ONE
                        # accumulation group (start only on the first) -- each
                        # start_tensor_calc zeroes the whole 2KB zero region on
                        # hardware, wiping earlier writes in the bank.
                        tp = tp_ps.tile([P, 4, P], BF16, tag="tp")
                        for i in range(4):
                            nc.tensor.matmul(
                                tp[:, i, :],
                                xrows[r][:, (g * 4 + i) * P:(g * 4 + i + 1) * P],
                                identity, is_transpose=True,
                                start=(i == 0), stop=(i == 3))
                        dst = xt_sb[:, g * 4:(g + 1) * 4, t0:t0 + P]
                        if copy_flip & 1:
                            nc.vector.tensor_copy(dst, tp[:])
                        else:
                            nc.scalar.activation(dst, tp[:], AF.Copy)
                        copy_flip += 1
                        next(kv_fill, None)
                        next(kv_fill, None)
                        next(kv_fill, None)
                kv_fill = kv_emit(c)
            for _ in kv_fill:
                pass

        # prefetch Wo
        nc.sync.dma_start(wo_sb[:], wo_v)

        qproj_ps = ctx.enter_context(
            tc.tile_pool(name="qproj_ps", bufs=1, space="PSUM"))
        wo_ps = ctx.enter_context(tc.tile_pool(name="wo_ps", bufs=1, space="PSUM"))
        s_ps = ctx.enter_context(tc.tile_pool(name="s_ps", bufs=2, space="PSUM"))
        pv_ps = ctx.enter_context(tc.tile_pool(name="pv_ps", bufs=2, space="PSUM"))

        # ---- Phase B: software-pipelined attention ----
        # q heads are permuted host-side to order [0,4,1,5,2,6,3,7] so that
        # head h sits at (block h%4, partition offset 64*(h//4)) -- the
        # partition offset then always equals its kv head's offset in kt_sb,
        # satisfying matmul's equal-base-partition requirement.
        #
        # The PE executes its queue in order, so any instruction that waits
        # stalls everything behind it AND drops the clock out of the ramped
        # p-state.  Three measures keep the PE stream gapless:
        #   - PV matmuls lag their exp group by one full group, covering the
        #     ~1.2us ACT exp latency;
        #   - the Wo matmuls of block tb-1 and the q^T projection of block
        #     tb+1 are interleaved as ACT-independent fillers (2 per group);
        #   - normalization (reciprocal/broadcast/mul) runs on DVE/Pool a
        #     full block ahead of the Wo matmuls that consume its output.

        def qproj_emit(tb):
            """q^T for block tb (scale pre-folded into Wq); yields per matmul."""
            for mb in range(DQ // P):
                qp = qproj_ps.tile([P, CH], F32, tag="qproj")
                for ks in range(NKS):
                    nc.tensor.matmul(
                        qp[:], wq_sb[:, ks, mb * P:(mb + 1) * P],
                        xt_sb[:, ks, tb * CH:(tb + 1) * CH],
                        start=(ks == 0), stop=(ks == NKS - 1))
                    yield
                nc.vector.tensor_copy(qt_sb[:, mb, tb * CH:(tb + 1) * CH], qp[:])

        def wo_emit(tb, outt, pools=None):
            """y = outt.T @ Wo for block tb; yields per matmul."""
            if pools is None:
                pools = ((wo_ps, "wo"),)
            for mi in range(TQB // P):
                y_sb = y_p.tile([P, D], F32, tag="y")
                for nb in range(NB):
                    pl, ptag = pools[nb % len(pools)]
                    yp = pl.tile([P, 512], F32, tag=ptag)
                    for ks in range(DQ // P):
                        nc.tensor.matmul(
                            yp[:], outt[:, ks, mi * P:(mi + 1) * P],
                            wo_sb[:, ks, nb * 512:(nb + 1) * 512],
                            start=(ks == 0), stop=(ks == DQ // P - 1))
                        yield
                    nc.vector.tensor_copy(y_sb[:, nb * 512:(nb + 1) * 512], yp[:])
                nc.sync.dma_start(y_v[:, tb * (TQB // P) + mi, :], y_sb[:])

        def normalize(pv, outt, h):
            mbq, poq = h % 4, (h // GROUP) * DH
            # reciprocal_approx_fast mislowers a non-zero input partition
            # base on HW -- stage the denominator row to partition 0 first
            den = rc_p.tile([1, TQB], F32, tag="den")
            nc.vector.tensor_copy(den[:], pv[DH:DH + 1, :])
            rc = rc_p.tile([1, TQB], F32, tag="rc")
            nc.vector.reciprocal_approx_fast(rc[:], den[:])
            bc = bc_p.tile([DH, TQB], F32, tag="bc")
            nc.gpsimd.partition_broadcast(bc[:], rc[:], channels=DH)
            nc.vector.tensor_mul(outt[poq:poq + DH, mbq, :], pv[0:DH, :], bc[:])

        # q^T for block 0 runs unpipelined ahead of the first attention block
        for _ in qproj_emit(0):
            pass

        NG = NKI // EG               # exp groups per head
        prev_outt = None
        for tb in range(NTQB):
            fillers = []
            if prev_outt is not None:
                fillers.append(wo_emit(tb - 1, prev_outt))
            if tb + 1 < NTQB:
                fillers.append(qproj_emit(tb + 1))
            filler = (f for gen in fillers for f in gen)

            outt_tb = ot_p.tile([P, DQ // P, TQB], BF16, tag="ot")
            lag = []                 # deferred PV groups: (pv, ex, j, g)
            pv = None
            for G in range(NHQ * NG):
                h, g = divmod(G, NG)
                j = h // GROUP       # kv head on this core
                mbq, poq = h % 4, (h // GROUP) * DH
                if g == 0:
                    pv = pv_ps.tile([DH + 1, TQB], F32, tag="pv")
                sp = s_ps.tile([P, EG, TQB], F32, tag="s")
                for i in range(EG):
                    ki = g * EG + i
                    nc.tensor.matmul(
                        sp[:, i, :],
                        kt_sb[j * DH:(j + 1) * DH, ki * P:(ki + 1) * P],
                        qt_sb[poq:poq + DH, mbq, tb * TQB:(tb + 1) * TQB],
                        start=True, stop=True)
                ex = exp_p.tile([P, EG, TQB], BF16, tag="exp")
                nc.scalar.activation(ex[:], sp[:], AF.Exp)
                lag.append((pv, ex, h, g))
                if len(lag) > 1:
                    lpv, lex, lh, lg = lag.pop(0)
                    for i in range(EG):
                        ki = lg * EG + i
                        nc.tensor.matmul(lpv[:], vones[:, lh // GROUP, ki, :],
                                         lex[:, i, :],
                                         start=(ki == 0), stop=(ki == NKI - 1))
                    if lg == NG - 1:
                        normalize(lpv, outt_tb, lh)
                for _ in range(2):
                    next(filler, None)
            # drain: last PV group + its normalize, then remaining fillers
            lpv, lex, lh, lg = lag.pop(0)
            for i in range(EG):
                ki = lg * EG + i
                nc.tensor.matmul(lpv[:], vones[:, lh // GROUP, ki, :], lex[:, i, :],
                                 start=(ki == 0), stop=(ki == NKI - 1))
            normalize(lpv, outt_tb, lh)
            for _ in filler:
                pass
            prev_outt = outt_tb

        # final Wo block runs unpipelined after the last attention block;
        # alternate two PSUM pools so evictions never stall the PE
        for _ in wo_emit(NTQB - 1, prev_outt,
                         pools=((wo_ps, "wo"), (qproj_ps, "qproj"))):
            pass

    nc.compile()
    return nc


_NC_CACHE = {}


def _get_nc():
    if "nc" not in _NC_CACHE:
        _NC_CACHE["nc"] = _build()
    return _NC_CACHE["nc"]


def make_in_maps(inputs):
    x = np.asarray(inputs["x"], dtype=np.float32)
    Wq = np.asarray(inputs["Wq"], dtype=np.float32) * SCALE
    Wk = np.asarray(inputs["Wk"], dtype=np.float32)
    Wv = np.asarray(inputs["Wv"], dtype=np.float32)
    Wo = np.asarray(inputs["Wo"], dtype=np.float32)
    # interleave the per-core q heads as [0,4,1,5,2,6,3,7] (see phase B note)
    perm = np.concatenate(
        [np.r_[b * DH:(b + 1) * DH, (b + 4) * DH:(b + 5) * DH] for b in range(4)])
    xb = [np.ascontiguousarray(x[b].astype(NPBF16)) for b in range(B)]
    in_maps = []
    for c in range(NCORES):
        b, g = divmod(c, TP)
        in_maps.append({
            "x": xb[b],
            "wq": np.ascontiguousarray(
                Wq[:, g * DQ:(g + 1) * DQ][:, perm].astype(NPBF16)),
            "wk": np.ascontiguousarray(
                Wk[:, g * DKV:(g + 1) * DKV].astype(NPBF16)),
            "wv": np.ascontiguousarray(
                Wv[:, g * DKV:(g + 1) * DKV].astype(NPBF16)),
            "wo": np.ascontiguousarray(
                Wo[g * DQ:(g + 1) * DQ, :][perm, :].astype(NPBF16)),
        })
    return in_maps


def kernel(x, Wq, Wk, Wv, Wo):
    nc = _get_nc()
    in_maps = make_in_maps({"x": x, "Wq": Wq, "Wk": Wk, "Wv": Wv, "Wo": Wo})
    res = run_bass_kernel_spmd(nc, in_maps, list(range(NCORES)))
    y = np.zeros((B, T, D), dtype=np.float32)
    for c in range(NCORES):
        b = c // TP
        y[b] += res.results[c]["y"]
    return y


# revision 17
# speedup vs baseline: 1.8232x; 1.0171x over previous
"""GQA attention block on 8 NeuronCores.

Sharding: tensor-parallel over head groups (4 ways: 8 q heads / 2 kv heads
per core) x data-parallel over batch (2 ways).  Each core computes a partial
y = attn_out_slice @ Wo_slice for its (batch, head-group); the host sums the
4 TP partials per batch element.

All matmul operands are bf16 (host-converted): fp32r matmuls at sustained
full rate trip the power throttle (~50% util limit for most of the run in
the v0 trace); bf16 streams at the same 1 col/cycle without throttling and
halves DMA + weight-load traffic.  PSUM accumulation stays fp32.

Per-core device program:
  A) x^T via PE transposes (grouped 4-per-PSUM-bank, single strided
     eviction copy alternating DVE/ACT); k^T/v^T projections per chunk.
  B) per T_q block: q^T projection for that chunk, then attention:
     S^T tiles = k^T.T @ q^T (scale folded into Wq host-side), exp on ACT
     batched over 2 PSUM banks, PV via lhsT=[v|ones] (row 64 = softmax
     denominator), normalize with reciprocal_approx_fast + partition
     broadcast + multiply.  Wo matmuls for block tb overlap block tb+1.
"""

import os
import sys

import numpy as np

for _p in ("/opt/trn_rl_repo",):
    if os.path.isdir(_p) and _p not in sys.path:
        sys.path.insert(0, _p)

from contextlib import ExitStack

import ml_dtypes

import concourse.bass as bass  # noqa: F401  (AP types pulled in transitively)
import concourse.mybir as mybir
import concourse.tile as tile
from concourse import bacc
from concourse.bass_utils import run_bass_kernel_spmd
from concourse.masks import make_identity

P = 128
B, T, D = 2, 2048, 2048
HQ, HKV, DH = 32, 8, 64
GROUP = HQ // HKV            # 4
TP = 4                       # tensor-parallel ways
DP = 2                       # data-parallel ways
NCORES = TP * DP
DQ = D // TP                 # 512 q dims per core (8 heads)
DKV = HKV * DH // TP         # 128 kv dims per core (2 kv heads)
NHQ = HQ // TP               # 8 q heads per core
NKV = HKV // TP              # 2 kv heads per core
NKS = D // P                 # 16 contraction subtiles over D
CH = 512                     # T chunk width in projection phase
NCH = T // CH                # 4
TQB = 512                    # T_q block width in attention / psum bank
NTQB = T // TQB              # 4
NKI = T // P                 # 16 key tiles
EG = 2                       # ki tiles per exp group (PSUM banks per S tile)
NB = D // 512                # 4 output column banks
SCALE = 1.0 / 8.0            # 1/sqrt(DH), folded into Wq on the host
F32 = mybir.dt.float32
BF16 = mybir.dt.bfloat16
AF = mybir.ActivationFunctionType
NPBF16 = np.dtype(ml_dtypes.bfloat16)


def _build():
    nc = bacc.Bacc(None, target_bir_lowering=False, debug=False)

    x_ext = nc.dram_tensor("x", [T, D], BF16, kind="ExternalInput")
    wq_ext = nc.dram_tensor("wq", [D, DQ], BF16, kind="ExternalInput")
    wk_ext = nc.dram_tensor("wk", [D, DKV], BF16, kind="ExternalInput")
    wv_ext = nc.dram_tensor("wv", [D, DKV], BF16, kind="ExternalInput")
    wo_ext = nc.dram_tensor("wo", [DQ, D], BF16, kind="ExternalInput")
    y_ext = nc.dram_tensor("y", [T, D], F32, kind="ExternalOutput")

    x_v = x_ext[:].rearrange("(to p) d -> p to d", p=P)      # [128,16,2048]
    wq_v = wq_ext[:].rearrange("(ko p) m -> p ko m", p=P)    # [128,16,512]
    wk_v = wk_ext[:].rearrange("(ko p) m -> p ko m", p=P)    # [128,16,128]
    wv_v = wv_ext[:].rearrange("(ko p) m -> p ko m", p=P)
    wo_v = wo_ext[:].rearrange("(ko p) n -> p ko n", p=P)    # [128,4,2048]
    y_v = y_ext[:].rearrange("(to p) n -> p to n", p=P)      # [128,16,2048]

    with tile.TileContext(nc) as tc, ExitStack() as ctx:
        const = ctx.enter_context(tc.tile_pool(name="const", bufs=1))
        w_p = ctx.enter_context(tc.tile_pool(name="wp", bufs=1))
        xt_p = ctx.enter_context(tc.tile_pool(name="xtp", bufs=1))
        row_p = ctx.enter_context(tc.tile_pool(name="rows", bufs=6))
        vt_p = ctx.enter_context(tc.tile_pool(name="vtp", bufs=2))
        qt_p = ctx.enter_context(tc.tile_pool(name="qt", bufs=1))
        kt_p = ctx.enter_context(tc.tile_pool(name="kt", bufs=1))
        vo_p = ctx.enter_context(tc.tile_pool(name="vo", bufs=1))
        exp_p = ctx.enter_context(tc.tile_pool(name="expp", bufs=3))
        bc_p = ctx.enter_context(tc.tile_pool(name="bcp", bufs=2))
        rc_p = ctx.enter_context(tc.tile_pool(name="rcp", bufs=2))
        ot_p = ctx.enter_context(tc.tile_pool(name="otp", bufs=2))
        y_p = ctx.enter_context(tc.tile_pool(name="yp", bufs=2))

        identity = const.tile([P, P], BF16)
        make_identity(nc, identity)

        wq_sb = w_p.tile([P, NKS, DQ], BF16)
        wk_sb = w_p.tile([P, NKS, DKV], BF16)
        wv_sb = w_p.tile([P, NKS, DKV], BF16)
        wo_sb = w_p.tile([P, DQ // P, D], BF16)

        xt_sb = xt_p.tile([P, NKS, T], BF16)            # x^T, [d%128, d//128, t]
        qt_sb = qt_p.tile([P, DQ // P, T], BF16)        # q^T * SCALE, [dim, t]
        kt_sb = kt_p.tile([P, T], BF16)                 # k^T, [dim(2 kv heads), t]
        vones = vo_p.tile([P, NKV, NKI, DH + 1], BF16)  # [t%128, kv, t//128, dh|1]
        ones_col = const.tile([P, NKV, NKI], BF16)
        nc.gpsimd.memset(ones_col[:], 1.0)
        nc.vector.tensor_copy(vones[:, :, :, DH], ones_col[:])

        # ---- Phase A: x^T + k^T/v^T projections, chunk by chunk ----
        with tc.tile_pool(name="tp_ps", bufs=4, space="PSUM") as tp_ps, \
             tc.tile_pool(name="proj_ps", bufs=2, space="PSUM") as proj_ps:

            def kv_emit(c):
                """k^T/v^T projections + v transpose for chunk c; yields per
                PE instruction so it can interleave with the next chunk's
                (DMA-gated) x transposes."""
                kp = proj_ps.tile([P, CH], F32, tag="proj")
                for ks in range(NKS):
                    nc.tensor.matmul(kp[:], wk_sb[:, ks, :],
                                     xt_sb[:, ks, c * CH:(c + 1) * CH],
                                     start=(ks == 0), stop=(ks == NKS - 1))
                    yield
                nc.scalar.activation(kt_sb[:, c * CH:(c + 1) * CH], kp[:], AF.Copy)
                vp = proj_ps.tile([P, CH], F32, tag="proj")
                for ks in range(NKS):
                    nc.tensor.matmul(vp[:], wv_sb[:, ks, :],
                                     xt_sb[:, ks, c * CH:(c + 1) * CH],
                                     start=(ks == 0), stop=(ks == NKS - 1))
                    yield
                vt_sb = vt_p.tile([P, CH], BF16, tag="vt")
                nc.scalar.activation(vt_sb[:], vp[:], AF.Copy)
                tpv = tp_ps.tile([P, 4, P], BF16, tag="tp")
                for r in range(CH // P):
                    nc.tensor.matmul(tpv[:, r, :],
                                     vt_sb[:, r * P:(r + 1) * P],
                                     identity, is_transpose=True,
                                     start=(r == 0), stop=(r == 3))
                    yield
                for r in range(CH // P):
                    ki = c * (CH // P) + r
                    for j in range(NKV):
                        nc.scalar.activation(vones[:, j, ki, 0:DH],
                                             tpv[:, r, j * DH:(j + 1) * DH],
                                             AF.Copy)

            copy_flip = 0
            kv_fill = iter(())
            for c in range(NCH):
                xrows = []
                for r in range(CH // P):
                    xrow = row_p.tile([P, D], BF16, tag="rows")
                    nc.sync.dma_start(xrow[:], x_v[:, c * (CH // P) + r, :])
                    xrows.append(xrow)
                if c == 0:
                    # wk/wv are small and needed first (kv projections of
                    # chunk 0); wq is 2MB and not needed until qproj(0), so
                    # it queues after chunk 2's x rows to keep the x stream
                    # feeding the transposes
                    nc.sync.dma_start(wk_sb[:], wk_v)
                    nc.sync.dma_start(wv_sb[:], wv_v)
                if c == 2:
                    nc.sync.dma_start(wq_sb[:], wq_v)
                for r in range(CH // P):
                    t0 = (c * (CH // P) + r) * P
                    for g in range(NKS // 4):
                        # 4 transposes share one PSUM bank: they must form ONE
                        # accumulation group (start only on the first) -- each
                        # start_tensor_calc zeroes the whole 2KB zero region on
                        # hardware, wiping earlier writes in the bank.
                        tp = tp_ps.tile([P, 4, P], BF16, tag="tp")
                        for i in range(4):
                            nc.tensor.matmul(
                                tp[:, i, :],
                                xrows[r][:, (g * 4 + i) * P:(g * 4 + i + 1) * P],
                                identity, is_transpose=True,
                                start=(i == 0), stop=(i == 3))
                        dst = xt_sb[:, g * 4:(g + 1) * 4, t0:t0 + P]
                        if copy_flip & 1:
                            nc.vector.tensor_copy(dst, tp[:])
                        else:
                            nc.scalar.activation(dst, tp[:], AF.Copy)
                        copy_flip += 1
                        next(kv_fill, None)
                        next(kv_fill, None)
                        next(kv_fill, None)
                kv_fill = kv_emit(c)
            for _ in kv_fill:
                pass

        # prefetch Wo
        nc.sync.dma_start(wo_sb[:], wo_v)

        qproj_ps = ctx.enter_context(
            tc.tile_pool(name="qproj_ps", bufs=1, space="PSUM"))
        wo_ps = ctx.enter_context(tc.tile_pool(name="wo_ps", bufs=1, space="PSUM"))
        s_ps = ctx.enter_context(tc.tile_pool(name="s_ps", bufs=2, space="PSUM"))
        pv_ps = ctx.enter_context(tc.tile_pool(name="pv_ps", bufs=2, space="PSUM"))

        # ---- Phase B: software-pipelined attention ----
        # q heads are permuted host-side to order [0,4,1,5,2,6,3,7] so that
        # head h sits at (block h%4, partition offset 64*(h//4)) -- the
        # partition offset then always equals its kv head's offset in kt_sb,
        # satisfying matmul's equal-base-partition requirement.
        #
        # The PE executes its queue in order, so any instruction that waits
        # stalls everything behind it AND drops the clock out of the ramped
        # p-state.  Three measures keep the PE stream gapless:
        #   - PV matmuls lag their exp group by one full group, covering the
        #     ~1.2us ACT exp latency;
        #   - the Wo matmuls of block tb-1 and the q^T projection of block
        #     tb+1 are interleaved as ACT-independent fillers (2 per group);
        #   - normalization (reciprocal/broadcast/mul) runs on DVE/Pool a
        #     full block ahead of the Wo matmuls that consume its output.

        def qproj_emit(tb):
            """q^T for block tb (scale pre-folded into Wq); yields per matmul."""
            for mb in range(DQ // P):
                qp = qproj_ps.tile([P, CH], F32, tag="qproj")
                for ks in range(NKS):
                    nc.tensor.matmul(
                        qp[:], wq_sb[:, ks, mb * P:(mb + 1) * P],
                        xt_sb[:, ks, tb * CH:(tb + 1) * CH],
                        start=(ks == 0), stop=(ks == NKS - 1))
                    yield
                nc.vector.tensor_copy(qt_sb[:, mb, tb * CH:(tb + 1) * CH], qp[:])

        def wo_emit(tb, outt, pools=None):
            """y = outt.T @ Wo for block tb; yields per matmul."""
            if pools is None:
                pools = ((wo_ps, "wo"),)
            for mi in range(TQB // P):
                y_sb = y_p.tile([P, D], F32, tag="y")
                for nb in range(NB):
                    pl, ptag = pools[nb % len(pools)]
                    yp = pl.tile([P, 512], F32, tag=ptag)
                    for ks in range(DQ // P):
                        nc.tensor.matmul(
                            yp[:], outt[:, ks, mi * P:(mi + 1) * P],
                            wo_sb[:, ks, nb * 512:(nb + 1) * 512],
                            start=(ks == 0), stop=(ks == DQ // P - 1))
                        yield
                    nc.vector.tensor_copy(y_sb[:, nb * 512:(nb + 1) * 512], yp[:])
                nc.sync.dma_start(y_v[:, tb * (TQB // P) + mi, :], y_sb[:])

        def normalize(pv, outt, h):
            mbq, poq = h % 4, (h // GROUP) * DH
            # reciprocal_approx_fast mislowers a non-zero input partition
            # base on HW -- stage the denominator row to partition 0 first
            den = rc_p.tile([1, TQB], F32, tag="den")
            nc.vector.tensor_copy(den[:], pv[DH:DH + 1, :])
            rc = rc_p.tile([1, TQB], F32, tag="rc")
            nc.vector.reciprocal_approx_fast(rc[:], den[:])
            bc = bc_p.tile([DH, TQB], F32, tag="bc")
            nc.gpsimd.partition_broadcast(bc[:], rc[:], channels=DH)
            nc.vector.tensor_mul(outt[poq:poq + DH, mbq, :], pv[0:DH, :], bc[:])

        # q^T for block 0 runs unpipelined ahead of the first attention block
        for _ in qproj_emit(0):
            pass

        NG = NKI // EG               # exp groups per head
        prev_outt = None
        for tb in range(NTQB):
            fillers = []
            if prev_outt is not None:
                fillers.append(wo_emit(tb - 1, prev_outt))
            if tb + 1 < NTQB:
                fillers.append(qproj_emit(tb + 1))
            filler = (f for gen in fillers for f in gen)

            outt_tb = ot_p.tile([P, DQ // P, TQB], BF16, tag="ot")
            lag = []                 # deferred PV groups: (pv, ex, j, g)
            pv = None
            for G in range(NHQ * NG):
                h, g = divmod(G, NG)
                j = h // GROUP       # kv head on this core
                mbq, poq = h % 4, (h // GROUP) * DH
                if g == 0:
                    pv = pv_ps.tile([DH + 1, TQB], F32, tag="pv")
                sp = s_ps.tile([P, EG, TQB], F32, tag="s")
                for i in range(EG):
                    ki = g * EG + i
                    nc.tensor.matmul(
                        sp[:, i, :],
                        kt_sb[j * DH:(j + 1) * DH, ki * P:(ki + 1) * P],
                        qt_sb[poq:poq + DH, mbq, tb * TQB:(tb + 1) * TQB],
                        start=True, stop=True)
                ex = exp_p.tile([P, EG, TQB], BF16, tag="exp")
                nc.scalar.activation(ex[:], sp[:], AF.Exp)
                lag.append((pv, ex, h, g))
                if len(lag) > 1:
                    lpv, lex, lh, lg = lag.pop(0)
                    for i in range(EG):
                        ki = lg * EG + i
                        nc.tensor.matmul(lpv[:], vones[:, lh // GROUP, ki, :],
                                         lex[:, i, :],
                                         start=(ki == 0), stop=(ki == NKI - 1))
                    if lg == NG - 1:
                        normalize(lpv, outt_tb, lh)
                for _ in range(2):
                    next(filler, None)
            # drain: last PV group + its normalize, then remaining fillers
            lpv, lex, lh, lg = lag.pop(0)
            for i in range(EG):
                ki = lg * EG + i
                nc.tensor.matmul(lpv[:], vones[:, lh // GROUP, ki, :], lex[:, i, :],
                                 start=(ki == 0), stop=(ki == NKI - 1))
            normalize(lpv, outt_tb, lh)
            for _ in filler:
                pass
            prev_outt = outt_tb

        # final Wo block runs unpipelined after the last attention block;
        # alternate two PSUM pools so evictions never stall the PE
        for _ in wo_emit(NTQB - 1, prev_outt,
                         pools=((wo_ps, "wo"), (qproj_ps, "qproj"))):
            pass

    nc.compile()
    return nc


_NC_CACHE = {}


def _get_nc():
    if "nc" not in _NC_CACHE:
        _NC_CACHE["nc"] = _build()
    return _NC_CACHE["nc"]


def make_in_maps(inputs):
    x = np.asarray(inputs["x"], dtype=np.float32)
    Wq = np.asarray(inputs["Wq"], dtype=np.float32) * SCALE
    Wk = np.asarray(inputs["Wk"], dtype=np.float32)
    Wv = np.asarray(inputs["Wv"], dtype=np.float32)
    Wo = np.asarray(inputs["Wo"], dtype=np.float32)
    # interleave the per-core q heads as [0,4,1,5,2,6,3,7] (see phase B note)
    perm = np.concatenate(
        [np.r_[b * DH:(b + 1) * DH, (b + 4) * DH:(b + 5) * DH] for b in range(4)])
    xb = [np.ascontiguousarray(x[b].astype(NPBF16)) for b in range(B)]
    in_maps = []
    for c in range(NCORES):
        b, g = divmod(c, TP)
        in_maps.append({
            "x": xb[b],
            "wq": np.ascontiguousarray(
                Wq[:, g * DQ:(g + 1) * DQ][:, perm].astype(NPBF16)),
            "wk": np.ascontiguousarray(
                Wk[:, g * DKV:(g + 1) * DKV].astype(NPBF16)),
            "wv": np.ascontiguousarray(
                Wv[:, g * DKV:(g + 1) * DKV].astype(NPBF16)),
            "wo": np.ascontiguousarray(
                Wo[g * DQ:(g + 1) * DQ, :][perm, :].astype(NPBF16)),
        })
    return in_maps


def kernel(x, Wq, Wk, Wv, Wo):
    nc = _get_nc()
    in_maps = make_in_maps({"x": x, "Wq": Wq, "Wk": Wk, "Wv": Wv, "Wo": Wo})
    res = run_bass_kernel_spmd(nc, in_maps, list(range(NCORES)))
    y = np.zeros((B, T, D), dtype=np.float32)
    for c in range(NCORES):
        b = c // TP
        y[b] += res.results[c]["y"]
    return y
